# revision 31
# baseline (speedup 1.0000x reference)
"""ComplEx edge-scoring kernel for Trainium2 (8 NeuronCores, SPMD).

score[e] = Re(<h, r, conj(t)>) for 400k edges gathered from node/rel
embedding tables.

Gather strategy (the crux): the only fast gather primitive on this stack is
the ANT `dma_gather` ucode, whose indices are int16 (< 32768). Node ids go to
100k, so edges are bucketed host-side by (src%4, dst%4) into 16 classes; a
class-(a,b) bucket gathers src rows from a strided view of the node table
(base = a rows, stride = 4 rows) with idx16 = src>>2, and dst rows likewise.
Rel ids (<1000) fit int16 directly.

Host-side, all 400k edges are bucketed globally and dealt round-robin to the
8 cores, so every core gets an identical compile-time layout: 16 buckets
padded to BUCKET_CAP edges each. Scores are un-permuted host-side.

Device layout: gathered edge ordinal n (within a chunk) lands on partition
n%128, block n//128 (dma_gather's native layout). Each chunk = CHUNK edges =
BLOCKS blocks of 128. Per chunk and role one (occasionally two, at bucket
boundaries) dma_gather ops fetch [128, BLOCKS, 256] f32 of embedding rows.

Compute per chunk, with r/i = first/second 128 features:
    pt = [rr*rt | ir*it]          (full-width mult)
    qt = [rr*it | ir*rt]          (two half-width mults)
    ct = [pt.r + pt.i | qt.r - qt.i]
    score_block = sum(h_block * ct_block)   (fused tensor_tensor_reduce)
"""

import sys

if "/opt/trn_rl_repo" not in sys.path:
    sys.path.insert(0, "/opt/trn_rl_repo")

from contextlib import ExitStack

import numpy as np

import concourse.bass as bass
import concourse.tile as tile
from concourse import bacc, mybir
from concourse.bass_utils import run_bass_kernel_spmd

N_NODES = 100000
N_RELS = 1000
N_EDGES = 400000
D = 256
HALF = D // 2
P = 128
NCORES = 8

BUCKET_CAP = 3328        # per-core per-bucket slots (multiple of 128)
NBUCKETS = 16
TOTAL = NBUCKETS * BUCKET_CAP   # 53248 padded edges per core
CHUNK = 2048
BLOCKS = CHUNK // P      # 16
NCH = TOTAL // CHUNK     # 26
NCOLS = TOTAL // P       # 416
NQUEUES = 4              # SWDGE queues = Q7 cpu pairs generating descriptors

F32 = mybir.dt.float32
BF16 = mybir.dt.bfloat16
I16 = mybir.dt.int16

mult = mybir.AluOpType.mult
add = mybir.AluOpType.add
sub = mybir.AluOpType.subtract


def emit_kernel(ctx, tc, table_hdl, idx_ap, scores_ap, *, n_nodes, n_rels,
                cap, chunk, nch, gather_bufs=2, work_bufs=2):
    nc = tc.nc
    blocks = chunk // P
    total = 16 * cap
    ncols = total // P
    qrows = n_nodes // 4

    ipool = ctx.enter_context(tc.tile_pool(name="idxp", bufs=1))
    gpool = ctx.enter_context(tc.tile_pool(name="gath", bufs=gather_bufs))
    wpool = ctx.enter_context(tc.tile_pool(name="work", bufs=work_bufs))
    apool = ctx.enter_context(tc.tile_pool(name="actp", bufs=1))
    spool = ctx.enter_context(tc.tile_pool(name="scor", bufs=1))

    idx_cols = total // 16
    idx_sb = ipool.tile([P, 3 * idx_cols], I16)
    nc.sync.dma_start(idx_sb[:], idx_ap)

    s_all = spool.tile([P, ncols], F32)

    def class_of(ordinal, role):
        if role == 0:
            return (ordinal // (4 * cap)) % 4
        if role == 1:
            return (ordinal // cap) % 4
        return None

    def splits(base, role):
        period = 4 * cap if role == 0 else (cap if role == 1 else total)
        out, s = [], base
        while s < base + chunk:
            nxt = min(base + chunk, (s // period + 1) * period)
            out.append((s, nxt - s, class_of(s, role)))
            s = nxt
        return out

    queue_rr = [0]

    def gather_chunk(c):
        tiles = []
        for role in range(3):
            g = gpool.tile([P, blocks * D], BF16, tag=f"g{role}")
            gv = g[:].rearrange("p (b d) -> p b d", d=D)
            for (start, n, cls) in splits(c * chunk, role):
                b0 = (start - c * chunk) // P
                nb = n // P
                if role == 2:
                    in_ap = bass.AP(table_hdl, n_nodes * D,
                                    [[D, n_rels], [1, D]])
                    estep = None
                else:
                    in_ap = bass.AP(table_hdl, cls * D,
                                    [[4 * D, qrows], [1, D]])
                    estep = 4 * D
                qn = queue_rr[0] % NQUEUES
                queue_rr[0] += 1
                nc.gpsimd.dma_gather(
                    out_ap=gv[:, b0:b0 + nb, :],
                    in_ap=in_ap,
                    idxs_ap=idx_sb[:, role * idx_cols + start // 16:
                                   role * idx_cols + (start + n) // 16],
                    num_idxs=n,
                    num_idxs_reg=n,
                    elem_size=D,
                    elem_step=estep,
                    single_packet=False,
                    queue_num=qn,
                )
            tiles.append(g)
        return tiles

    def halves(ap, blks):
        v = ap.rearrange("p (b two d) -> p b two d", two=2, d=HALF)
        return v[:, :, 0, :], v[:, :, 1, :]

    def compute_chunk(c, tiles):
        gh, gt, gr = (t[:] for t in tiles)
        kd = blocks * D
        t_r, t_i = halves(gt, blocks)
        r_r, r_i = halves(gr, blocks)
        pt = wpool.tile([P, kd], BF16, tag="pt")
        nc.vector.tensor_tensor(out=pt[:], in0=gt, in1=gr, op=mult)
        qt = wpool.tile([P, kd], BF16, tag="qt")
        q_r, q_i = halves(qt[:], blocks)
        nc.vector.tensor_tensor(out=q_r, in0=r_r, in1=t_i, op=mult)
        nc.vector.tensor_tensor(out=q_i, in0=r_i, in1=t_r, op=mult)
        ct = wpool.tile([P, kd], BF16, tag="ct")
        c_r, c_i = halves(ct[:], blocks)
        p_r, p_i = halves(pt[:], blocks)
        nc.vector.tensor_tensor(out=c_r, in0=p_r, in1=p_i, op=add)
        nc.vector.tensor_tensor(out=c_i, in0=q_r, in1=q_i, op=sub)
        prod = wpool.tile([P, kd], BF16, tag="pr")
        nc.vector.tensor_tensor(out=prod[:], in0=gh, in1=ct[:], op=mult)
        asc = apool.tile([P, kd], BF16, tag="as")
        for b in range(blocks):
            nc.scalar.activation(
                out=asc[:, b * D:(b + 1) * D],
                in_=prod[:, b * D:(b + 1) * D],
                func=mybir.ActivationFunctionType.Copy,
                accum_out=s_all[:, c * blocks + b:c * blocks + b + 1],
            )

    prefetch = min(gather_bufs - 1, nch - 1, 3)
    pending = [gather_chunk(i) for i in range(prefetch + 1)]
    for c in range(nch):
        cur = pending.pop(0)
        if c + prefetch + 1 < nch:
            pending.append(gather_chunk(c + prefetch + 1))
        compute_chunk(c, cur)

    nc.sync.dma_start(scores_ap, s_all[:])


def build_program(*, n_nodes=N_NODES, n_rels=N_RELS, cap=BUCKET_CAP,
                  chunk=CHUNK, num_devices=NCORES, gather_bufs=4,
                  work_bufs=2, enable_asserts=False, num_swdge_queues=NQUEUES):
    total = 16 * cap
    nch = total // chunk
    nc = bacc.Bacc(
        "TRN2",
        target_bir_lowering=False,
        debug=False,
        enable_asserts=enable_asserts,
        num_devices=num_devices,
        num_swdge_queues=num_swdge_queues,
    )
    table_h = nc.dram_tensor("table", [n_nodes + n_rels, D], BF16,
                             kind="ExternalInput")
    idx = nc.dram_tensor("idx16", [P, 3 * (total // 16)], I16,
                         kind="ExternalInput").ap()
    scores = nc.dram_tensor("scores", [P, total // P], F32,
                            kind="ExternalOutput").ap()
    with tile.TileContext(nc) as tc, ExitStack() as ctx:
        emit_kernel(ctx, tc, table_h, idx, scores, n_nodes=n_nodes,
                    n_rels=n_rels, cap=cap, chunk=chunk, nch=nch,
                    gather_bufs=gather_bufs, work_bufs=work_bufs)
    nc.compile()
    return nc


def _wrap16(vals):
    """[total] int16 -> [128, total/16] wrapped layout (ordinal n at
    partition n%16, col n//16; replicated across the 8 Q7 core groups)."""
    w = vals.reshape(-1, 16).T.astype(np.int16)
    return np.tile(w, (8, 1))


def pack_inputs(node_emb, rel_emb, src, dst, rel_id, *, cap=BUCKET_CAP):
    """Bucket + deal edges to cores; build per-core idx16 arrays.

    Returns (table, per_core_idx16 list, slot_edge list).
    """
    total = 16 * cap
    key = (src % 4) * 4 + (dst % 4)
    order = np.argsort(key, kind="stable")
    sorted_key = key[order]
    bucket_starts = np.searchsorted(sorted_key, np.arange(17))
    import ml_dtypes

    table = np.concatenate([node_emb, rel_emb], axis=0).astype(
        ml_dtypes.bfloat16)

    per_core_slots = []
    for core in range(NCORES):
        slots = np.empty(total, dtype=np.int64)
        for b in range(16):
            members = order[bucket_starts[b] + core:bucket_starts[b + 1]:NCORES]
            assert 0 < len(members) <= cap, (
                f"bucket {b} core {core}: {len(members)} > cap {cap}"
            )
            padded = np.full(cap, members[-1], dtype=np.int64)
            padded[:len(members)] = members
            slots[b * cap:(b + 1) * cap] = padded
        per_core_slots.append(slots)

    per_core_idx = []
    for slots in per_core_slots:
        s, d_, r = src[slots], dst[slots], rel_id[slots]
        idx16 = np.concatenate([
            _wrap16(s >> 2), _wrap16(d_ >> 2), _wrap16(r)], axis=1)
        per_core_idx.append(np.ascontiguousarray(idx16))
    return table, per_core_idx, per_core_slots


_PROGRAM_CACHE = {}

# test-harness hooks: test.py sets _RUN_KWARGS["trace"]=True and reads _LAST
_RUN_KWARGS = {}
_LAST = {}


def _get_program():
    if "nc" not in _PROGRAM_CACHE:
        _PROGRAM_CACHE["nc"] = build_program()
    return _PROGRAM_CACHE["nc"]


def kernel(node_emb, rel_emb, src, dst, rel_id):
    node_emb = np.ascontiguousarray(np.asarray(node_emb, dtype=np.float32))
    rel_emb = np.ascontiguousarray(np.asarray(rel_emb, dtype=np.float32))
    src = np.asarray(src).astype(np.int64)
    dst = np.asarray(dst).astype(np.int64)
    rel_id = np.asarray(rel_id).astype(np.int64)

    table, per_core_idx, per_core_slots = pack_inputs(
        node_emb, rel_emb, src, dst, rel_id)
    nc = _get_program()

    in_maps = [
        {"table": table, "idx16": per_core_idx[m]} for m in range(NCORES)
    ]
    res = run_bass_kernel_spmd(nc, in_maps, core_ids=list(range(NCORES)),
                               **_RUN_KWARGS)
    _LAST["res"] = res

    out = np.empty(N_EDGES, dtype=np.float32)
    for m in range(NCORES):
        scores_sorted = res.results[m]["scores"].T.reshape(-1)  # [TOTAL]
        out[per_core_slots[m]] = scores_sorted
    return out



# revision 33
# speedup vs baseline: 1.0487x; 1.0487x over previous
"""ComplEx edge-scoring kernel for Trainium2 (8 NeuronCores, SPMD).

score[e] = Re(<h, r, conj(t)>) for 400k edges gathered from node/rel
embedding tables.

Gather strategy (the crux): the only fast gather primitive on this stack is
the ANT `dma_gather` ucode, whose indices are int16 (< 32768). Node ids go to
100k, so edges are bucketed host-side by (src%4, dst%4) into 16 classes; a
class-(a,b) bucket gathers src rows from a strided view of the node table
(base = a rows, stride = 4 rows) with idx16 = src>>2, and dst rows likewise.
Rel ids (<1000) fit int16 directly.

Host-side, all 400k edges are bucketed globally and dealt round-robin to the
8 cores, so every core gets an identical compile-time layout: 16 buckets
padded to BUCKET_CAP edges each. Scores are un-permuted host-side.

Device layout: gathered edge ordinal n (within a chunk) lands on partition
n%128, block n//128 (dma_gather's native layout). Each chunk = CHUNK edges =
BLOCKS blocks of 128. Per chunk and role one (occasionally two, at bucket
boundaries) dma_gather ops fetch [128, BLOCKS, 256] f32 of embedding rows.

Compute per chunk, with r/i = first/second 128 features:
    pt = [rr*rt | ir*it]          (full-width mult)
    qt = [rr*it | ir*rt]          (two half-width mults)
    ct = [pt.r + pt.i | qt.r - qt.i]
    score_block = sum(h_block * ct_block)   (fused tensor_tensor_reduce)
"""

import sys

if "/opt/trn_rl_repo" not in sys.path:
    sys.path.insert(0, "/opt/trn_rl_repo")

from contextlib import ExitStack

import numpy as np

import concourse.bass as bass
import concourse.tile as tile
from concourse import bacc, mybir
from concourse.bass_utils import run_bass_kernel_spmd

N_NODES = 100000
N_RELS = 1000
N_EDGES = 400000
D = 256
HALF = D // 2
P = 128
NCORES = 8

BUCKET_CAP = 3328        # per-core per-bucket slots (multiple of 128)
NBUCKETS = 16
TOTAL = NBUCKETS * BUCKET_CAP   # 53248 padded edges per core
CHUNK = 1024
BLOCKS = CHUNK // P      # 8
NCH = TOTAL // CHUNK     # 52
NCOLS = TOTAL // P       # 416
NQUEUES = 4              # SWDGE queues = Q7 cpu pairs generating descriptors

F32 = mybir.dt.float32
BF16 = mybir.dt.bfloat16
I16 = mybir.dt.int16

mult = mybir.AluOpType.mult
add = mybir.AluOpType.add
sub = mybir.AluOpType.subtract


def emit_kernel(ctx, tc, table_hdl, idx_ap, scores_ap, *, n_nodes, n_rels,
                cap, chunk, nch, gather_bufs=2, work_bufs=2):
    nc = tc.nc
    blocks = chunk // P
    total = 16 * cap
    ncols = total // P
    qrows = n_nodes // 4

    ipool = ctx.enter_context(tc.tile_pool(name="idxp", bufs=1))
    gpool = ctx.enter_context(tc.tile_pool(name="gath", bufs=gather_bufs))
    wpool = ctx.enter_context(tc.tile_pool(name="work", bufs=work_bufs))
    apool = ctx.enter_context(tc.tile_pool(name="actp", bufs=1))
    spool = ctx.enter_context(tc.tile_pool(name="scor", bufs=1))

    idx_cols = total // 16
    idx_sb = ipool.tile([P, 3 * idx_cols], I16)
    nc.sync.dma_start(idx_sb[:], idx_ap)

    s_all = spool.tile([P, ncols], F32)

    def class_of(ordinal, role):
        if role == 0:
            return (ordinal // (4 * cap)) % 4
        if role == 1:
            return (ordinal // cap) % 4
        return None

    def splits(base, role):
        period = 4 * cap if role == 0 else (cap if role == 1 else total)
        out, s = [], base
        while s < base + chunk:
            nxt = min(base + chunk, (s // period + 1) * period)
            out.append((s, nxt - s, class_of(s, role)))
            s = nxt
        return out

    queue_rr = [0]

    def gather_chunk(c):
        tiles = []
        for role in range(3):
            g = gpool.tile([P, blocks * D], BF16, tag=f"g{role}")
            gv = g[:].rearrange("p (b d) -> p b d", d=D)
            for (start, n, cls) in splits(c * chunk, role):
                b0 = (start - c * chunk) // P
                nb = n // P
                if role == 2:
                    in_ap = bass.AP(table_hdl, n_nodes * D,
                                    [[D, n_rels], [1, D]])
                    estep = None
                else:
                    in_ap = bass.AP(table_hdl, cls * D,
                                    [[4 * D, qrows], [1, D]])
                    estep = 4 * D
                qn = queue_rr[0] % NQUEUES
                queue_rr[0] += 1
                nc.gpsimd.dma_gather(
                    out_ap=gv[:, b0:b0 + nb, :],
                    in_ap=in_ap,
                    idxs_ap=idx_sb[:, role * idx_cols + start // 16:
                                   role * idx_cols + (start + n) // 16],
                    num_idxs=n,
                    num_idxs_reg=n,
                    elem_size=D,
                    elem_step=estep,
                    single_packet=False,
                    queue_num=qn,
                )
            tiles.append(g)
        return tiles

    def halves(ap, blks):
        v = ap.rearrange("p (b two d) -> p b two d", two=2, d=HALF)
        return v[:, :, 0, :], v[:, :, 1, :]

    def compute_chunk(c, tiles):
        gh, gt, gr = (t[:] for t in tiles)
        kd = blocks * D
        t_r, t_i = halves(gt, blocks)
        r_r, r_i = halves(gr, blocks)
        pt = wpool.tile([P, kd], BF16, tag="pt")
        nc.vector.tensor_tensor(out=pt[:], in0=gt, in1=gr, op=mult)
        qt = wpool.tile([P, kd], BF16, tag="qt")
        q_r, q_i = halves(qt[:], blocks)
        nc.vector.tensor_tensor(out=q_r, in0=r_r, in1=t_i, op=mult)
        nc.vector.tensor_tensor(out=q_i, in0=r_i, in1=t_r, op=mult)
        ct = wpool.tile([P, kd], BF16, tag="ct")
        c_r, c_i = halves(ct[:], blocks)
        p_r, p_i = halves(pt[:], blocks)
        nc.vector.tensor_tensor(out=c_r, in0=p_r, in1=p_i, op=add)
        nc.vector.tensor_tensor(out=c_i, in0=q_r, in1=q_i, op=sub)
        prod = wpool.tile([P, kd], BF16, tag="pr")
        nc.vector.tensor_tensor(out=prod[:], in0=gh, in1=ct[:], op=mult)
        asc = apool.tile([P, kd], BF16, tag="as")
        for b in range(blocks):
            nc.scalar.activation(
                out=asc[:, b * D:(b + 1) * D],
                in_=prod[:, b * D:(b + 1) * D],
                func=mybir.ActivationFunctionType.Copy,
                accum_out=s_all[:, c * blocks + b:c * blocks + b + 1],
            )

    prefetch = min(gather_bufs - 1, nch - 1, 3)
    pending = [gather_chunk(i) for i in range(prefetch + 1)]
    for c in range(nch):
        cur = pending.pop(0)
        if c + prefetch + 1 < nch:
            pending.append(gather_chunk(c + prefetch + 1))
        compute_chunk(c, cur)

    nc.sync.dma_start(scores_ap, s_all[:])


def build_program(*, n_nodes=N_NODES, n_rels=N_RELS, cap=BUCKET_CAP,
                  chunk=CHUNK, num_devices=NCORES, gather_bufs=5,
                  work_bufs=2, enable_asserts=False, num_swdge_queues=NQUEUES):
    total = 16 * cap
    nch = total // chunk
    nc = bacc.Bacc(
        "TRN2",
        target_bir_lowering=False,
        debug=False,
        enable_asserts=enable_asserts,
        num_devices=num_devices,
        num_swdge_queues=num_swdge_queues,
    )
    table_h = nc.dram_tensor("table", [n_nodes + n_rels, D], BF16,
                             kind="ExternalInput")
    idx = nc.dram_tensor("idx16", [P, 3 * (total // 16)], I16,
                         kind="ExternalInput").ap()
    scores = nc.dram_tensor("scores", [P, total // P], F32,
                            kind="ExternalOutput").ap()
    with tile.TileContext(nc) as tc, ExitStack() as ctx:
        emit_kernel(ctx, tc, table_h, idx, scores, n_nodes=n_nodes,
                    n_rels=n_rels, cap=cap, chunk=chunk, nch=nch,
                    gather_bufs=gather_bufs, work_bufs=work_bufs)
    nc.compile()
    return nc


def _wrap16(vals):
    """[total] int16 -> [128, total/16] wrapped layout (ordinal n at
    partition n%16, col n//16; replicated across the 8 Q7 core groups)."""
    w = vals.reshape(-1, 16).T.astype(np.int16)
    return np.tile(w, (8, 1))


def pack_inputs(node_emb, rel_emb, src, dst, rel_id, *, cap=BUCKET_CAP):
    """Bucket + deal edges to cores; build per-core idx16 arrays.

    Returns (table, per_core_idx16 list, slot_edge list).
    """
    total = 16 * cap
    key = (src % 4) * 4 + (dst % 4)
    order = np.argsort(key, kind="stable")
    sorted_key = key[order]
    bucket_starts = np.searchsorted(sorted_key, np.arange(17))
    import ml_dtypes

    table = np.concatenate([node_emb, rel_emb], axis=0).astype(
        ml_dtypes.bfloat16)

    per_core_slots = []
    for core in range(NCORES):
        slots = np.empty(total, dtype=np.int64)
        for b in range(16):
            members = order[bucket_starts[b] + core:bucket_starts[b + 1]:NCORES]
            assert 0 < len(members) <= cap, (
                f"bucket {b} core {core}: {len(members)} > cap {cap}"
            )
            padded = np.full(cap, members[-1], dtype=np.int64)
            padded[:len(members)] = members
            slots[b * cap:(b + 1) * cap] = padded
        per_core_slots.append(slots)

    per_core_idx = []
    for slots in per_core_slots:
        s, d_, r = src[slots], dst[slots], rel_id[slots]
        idx16 = np.concatenate([
            _wrap16(s >> 2), _wrap16(d_ >> 2), _wrap16(r)], axis=1)
        per_core_idx.append(np.ascontiguousarray(idx16))
    return table, per_core_idx, per_core_slots


_PROGRAM_CACHE = {}

# test-harness hooks: test.py sets _RUN_KWARGS["trace"]=True and reads _LAST
_RUN_KWARGS = {}
_LAST = {}


def _get_program():
    if "nc" not in _PROGRAM_CACHE:
        _PROGRAM_CACHE["nc"] = build_program()
    return _PROGRAM_CACHE["nc"]


def kernel(node_emb, rel_emb, src, dst, rel_id):
    node_emb = np.ascontiguousarray(np.asarray(node_emb, dtype=np.float32))
    rel_emb = np.ascontiguousarray(np.asarray(rel_emb, dtype=np.float32))
    src = np.asarray(src).astype(np.int64)
    dst = np.asarray(dst).astype(np.int64)
    rel_id = np.asarray(rel_id).astype(np.int64)

    table, per_core_idx, per_core_slots = pack_inputs(
        node_emb, rel_emb, src, dst, rel_id)
    nc = _get_program()

    in_maps = [
        {"table": table, "idx16": per_core_idx[m]} for m in range(NCORES)
    ]
    res = run_bass_kernel_spmd(nc, in_maps, core_ids=list(range(NCORES)),
                               **_RUN_KWARGS)
    _LAST["res"] = res

    out = np.empty(N_EDGES, dtype=np.float32)
    for m in range(NCORES):
        scores_sorted = res.results[m]["scores"].T.reshape(-1)  # [TOTAL]
        out[per_core_slots[m]] = scores_sorted
    return out



# revision 34
# speedup vs baseline: 1.0606x; 1.0114x over previous
"""ComplEx edge-scoring kernel for Trainium2 (8 NeuronCores, SPMD).

score[e] = Re(<h, r, conj(t)>) for 400k edges gathered from node/rel
embedding tables.

Gather strategy (the crux): the only fast gather primitive on this stack is
the ANT `dma_gather` ucode, whose indices are int16 (< 32768). Node ids go to
100k, so edges are bucketed host-side by (src%4, dst%4) into 16 classes; a
class-(a,b) bucket gathers src rows from a strided view of the node table
(base = a rows, stride = 4 rows) with idx16 = src>>2, and dst rows likewise.
Rel ids (<1000) fit int16 directly.

Host-side, all 400k edges are bucketed globally and dealt round-robin to the
8 cores, so every core gets an identical compile-time layout: 16 buckets
padded to BUCKET_CAP edges each. Scores are un-permuted host-side.

Device layout: gathered edge ordinal n (within a chunk) lands on partition
n%128, block n//128 (dma_gather's native layout). Each chunk = CHUNK edges =
BLOCKS blocks of 128. Per chunk and role one (occasionally two, at bucket
boundaries) dma_gather ops fetch [128, BLOCKS, 256] f32 of embedding rows.

Compute per chunk, with r/i = first/second 128 features:
    pt = [rr*rt | ir*it]          (full-width mult)
    qt = [rr*it | ir*rt]          (two half-width mults)
    ct = [pt.r + pt.i | qt.r - qt.i]
    score_block = sum(h_block * ct_block)   (fused tensor_tensor_reduce)
"""

import sys

if "/opt/trn_rl_repo" not in sys.path:
    sys.path.insert(0, "/opt/trn_rl_repo")

from contextlib import ExitStack

import numpy as np

import concourse.bass as bass
import concourse.tile as tile
from concourse import bacc, mybir
from concourse.bass_utils import run_bass_kernel_spmd

N_NODES = 100000
N_RELS = 1000
N_EDGES = 400000
D = 256
HALF = D // 2
P = 128
NCORES = 8

BUCKET_CAP = 3328        # per-core per-bucket slots (multiple of 128)
NBUCKETS = 16
TOTAL = NBUCKETS * BUCKET_CAP   # 53248 padded edges per core
CHUNK = 1024
BLOCKS = CHUNK // P      # 8
NCH = TOTAL // CHUNK     # 52
NCOLS = TOTAL // P       # 416
NQUEUES = 4              # SWDGE queues = Q7 cpu pairs generating descriptors

F32 = mybir.dt.float32
BF16 = mybir.dt.bfloat16
I16 = mybir.dt.int16

mult = mybir.AluOpType.mult
add = mybir.AluOpType.add
sub = mybir.AluOpType.subtract


def emit_kernel(ctx, tc, table_hdl, idx_ap, scores_ap, *, n_nodes, n_rels,
                cap, chunk, nch, gather_bufs=2, work_bufs=2):
    nc = tc.nc
    blocks = chunk // P
    total = 16 * cap
    ncols = total // P
    qrows = n_nodes // 4

    ipool = ctx.enter_context(tc.tile_pool(name="idxp", bufs=1))
    gpool = ctx.enter_context(tc.tile_pool(name="gath", bufs=gather_bufs))
    wpool = ctx.enter_context(tc.tile_pool(name="work", bufs=work_bufs))
    apool = ctx.enter_context(tc.tile_pool(name="actp", bufs=1))
    spool = ctx.enter_context(tc.tile_pool(name="scor", bufs=1))

    idx_cols = total // 16
    idx_sb = ipool.tile([P, 3 * idx_cols], I16)
    nc.sync.dma_start(idx_sb[:], idx_ap)

    s_all = spool.tile([P, ncols], F32)

    def class_of(ordinal, role):
        if role == 0:
            return (ordinal // (4 * cap)) % 4
        if role == 1:
            return (ordinal // cap) % 4
        return None

    def splits(base, role):
        period = 4 * cap if role == 0 else (cap if role == 1 else total)
        out, s = [], base
        while s < base + chunk:
            nxt = min(base + chunk, (s // period + 1) * period)
            out.append((s, nxt - s, class_of(s, role)))
            s = nxt
        return out

    queue_rr = [0]

    def gather_chunk(c):
        tiles = []
        for role in range(3):
            g = gpool.tile([P, blocks * D], BF16, tag=f"g{role}")
            gv = g[:].rearrange("p (b d) -> p b d", d=D)
            for (start, n, cls) in splits(c * chunk, role):
                b0 = (start - c * chunk) // P
                nb = n // P
                if role == 2:
                    in_ap = bass.AP(table_hdl, n_nodes * D,
                                    [[D, n_rels], [1, D]])
                    estep = None
                else:
                    in_ap = bass.AP(table_hdl, cls * D,
                                    [[4 * D, qrows], [1, D]])
                    estep = 4 * D
                qn = queue_rr[0] % NQUEUES
                queue_rr[0] += 1
                nc.gpsimd.dma_gather(
                    out_ap=gv[:, b0:b0 + nb, :],
                    in_ap=in_ap,
                    idxs_ap=idx_sb[:, role * idx_cols + start // 16:
                                   role * idx_cols + (start + n) // 16],
                    num_idxs=n,
                    num_idxs_reg=n,
                    elem_size=D,
                    elem_step=estep,
                    single_packet=False,
                    queue_num=qn,
                )
            tiles.append(g)
        return tiles

    def halves(ap, blks):
        v = ap.rearrange("p (b two d) -> p b two d", two=2, d=HALF)
        return v[:, :, 0, :], v[:, :, 1, :]

    def compute_chunk(c, tiles):
        gh, gt, gr = (t[:] for t in tiles)
        kd = blocks * D
        t_r, t_i = halves(gt, blocks)
        r_r, r_i = halves(gr, blocks)
        pt = wpool.tile([P, kd], BF16, tag="pt")
        nc.vector.tensor_tensor(out=pt[:], in0=gt, in1=gr, op=mult)
        qt = wpool.tile([P, kd], BF16, tag="qt")
        q_r, q_i = halves(qt[:], blocks)
        nc.vector.tensor_tensor(out=q_r, in0=r_r, in1=t_i, op=mult)
        nc.vector.tensor_tensor(out=q_i, in0=r_i, in1=t_r, op=mult)
        ct = wpool.tile([P, kd], BF16, tag="ct")
        c_r, c_i = halves(ct[:], blocks)
        p_r, p_i = halves(pt[:], blocks)
        nc.vector.tensor_tensor(out=c_r, in0=p_r, in1=p_i, op=add)
        nc.vector.tensor_tensor(out=c_i, in0=q_r, in1=q_i, op=sub)
        prod = wpool.tile([P, kd], BF16, tag="pr")
        nc.vector.tensor_tensor(out=prod[:], in0=gh, in1=ct[:], op=mult)
        asc = apool.tile([P, kd], BF16, tag="as")
        for b in range(blocks):
            nc.scalar.activation(
                out=asc[:, b * D:(b + 1) * D],
                in_=prod[:, b * D:(b + 1) * D],
                func=mybir.ActivationFunctionType.Copy,
                accum_out=s_all[:, c * blocks + b:c * blocks + b + 1],
            )

    prefetch = min(gather_bufs - 1, nch - 1, 2)
    pending = [gather_chunk(i) for i in range(prefetch + 1)]
    for c in range(nch):
        cur = pending.pop(0)
        if c + prefetch + 1 < nch:
            pending.append(gather_chunk(c + prefetch + 1))
        compute_chunk(c, cur)

    nc.sync.dma_start(scores_ap, s_all[:])


def build_program(*, n_nodes=N_NODES, n_rels=N_RELS, cap=BUCKET_CAP,
                  chunk=CHUNK, num_devices=NCORES, gather_bufs=5,
                  work_bufs=2, enable_asserts=False, num_swdge_queues=NQUEUES):
    total = 16 * cap
    nch = total // chunk
    nc = bacc.Bacc(
        "TRN2",
        target_bir_lowering=False,
        debug=False,
        enable_asserts=enable_asserts,
        num_devices=num_devices,
        num_swdge_queues=num_swdge_queues,
    )
    table_h = nc.dram_tensor("table", [n_nodes + n_rels, D], BF16,
                             kind="ExternalInput")
    idx = nc.dram_tensor("idx16", [P, 3 * (total // 16)], I16,
                         kind="ExternalInput").ap()
    scores = nc.dram_tensor("scores", [P, total // P], F32,
                            kind="ExternalOutput").ap()
    with tile.TileContext(nc) as tc, ExitStack() as ctx:
        emit_kernel(ctx, tc, table_h, idx, scores, n_nodes=n_nodes,
                    n_rels=n_rels, cap=cap, chunk=chunk, nch=nch,
                    gather_bufs=gather_bufs, work_bufs=work_bufs)
    nc.compile()
    return nc


def _wrap16(vals):
    """[total] int16 -> [128, total/16] wrapped layout (ordinal n at
    partition n%16, col n//16; replicated across the 8 Q7 core groups)."""
    w = vals.reshape(-1, 16).T.astype(np.int16)
    return np.tile(w, (8, 1))


def pack_inputs(node_emb, rel_emb, src, dst, rel_id, *, cap=BUCKET_CAP):
    """Bucket + deal edges to cores; build per-core idx16 arrays.

    Returns (table, per_core_idx16 list, slot_edge list).
    """
    total = 16 * cap
    key = (src % 4) * 4 + (dst % 4)
    order = np.argsort(key, kind="stable")
    sorted_key = key[order]
    bucket_starts = np.searchsorted(sorted_key, np.arange(17))
    import ml_dtypes

    table = np.concatenate([node_emb, rel_emb], axis=0).astype(
        ml_dtypes.bfloat16)

    per_core_slots = []
    for core in range(NCORES):
        slots = np.empty(total, dtype=np.int64)
        for b in range(16):
            members = order[bucket_starts[b] + core:bucket_starts[b + 1]:NCORES]
            assert 0 < len(members) <= cap, (
                f"bucket {b} core {core}: {len(members)} > cap {cap}"
            )
            padded = np.full(cap, members[-1], dtype=np.int64)
            padded[:len(members)] = members
            slots[b * cap:(b + 1) * cap] = padded
        per_core_slots.append(slots)

    per_core_idx = []
    for slots in per_core_slots:
        s, d_, r = src[slots], dst[slots], rel_id[slots]
        idx16 = np.concatenate([
            _wrap16(s >> 2), _wrap16(d_ >> 2), _wrap16(r)], axis=1)
        per_core_idx.append(np.ascontiguousarray(idx16))
    return table, per_core_idx, per_core_slots


_PROGRAM_CACHE = {}

# test-harness hooks: test.py sets _RUN_KWARGS["trace"]=True and reads _LAST
_RUN_KWARGS = {}
_LAST = {}


def _get_program():
    if "nc" not in _PROGRAM_CACHE:
        _PROGRAM_CACHE["nc"] = build_program()
    return _PROGRAM_CACHE["nc"]


def kernel(node_emb, rel_emb, src, dst, rel_id):
    node_emb = np.ascontiguousarray(np.asarray(node_emb, dtype=np.float32))
    rel_emb = np.ascontiguousarray(np.asarray(rel_emb, dtype=np.float32))
    src = np.asarray(src).astype(np.int64)
    dst = np.asarray(dst).astype(np.int64)
    rel_id = np.asarray(rel_id).astype(np.int64)

    table, per_core_idx, per_core_slots = pack_inputs(
        node_emb, rel_emb, src, dst, rel_id)
    nc = _get_program()

    in_maps = [
        {"table": table, "idx16": per_core_idx[m]} for m in range(NCORES)
    ]
    res = run_bass_kernel_spmd(nc, in_maps, core_ids=list(range(NCORES)),
                               **_RUN_KWARGS)
    _LAST["res"] = res

    out = np.empty(N_EDGES, dtype=np.float32)
    for m in range(NCORES):
        scores_sorted = res.results[m]["scores"].T.reshape(-1)  # [TOTAL]
        out[per_core_slots[m]] = scores_sorted
    return out

